# revision 9
# baseline (speedup 1.0000x reference)
"""Embedding lookup (gather) on 8 TRN2 NeuronCores.

Strategy (per the row-sharding hint): the 1M x 128 table is row-sharded by
value range -- core c owns rows [c*125000, (c+1)*125000), held as 4 windows
of 31250 rows so window-local indices fit int16. The host routes each of the
500K indices to its owning window (the sharding step) and the device gathers
rows with batched-descriptor SWDGE gathers (InstDMAGatherAnt, ~0.34ns/row
descriptor) instead of one indirect DMA per 128 rows (~1.5us fixed cost
each). Unsharding re-assembles rows into token order on the host (inverse of
the routing permutation).

The table is fed to the device as bf16 (max rounding error 2^-8 ~ 0.4%,
well inside the 2e-2 gate) which halves both the gathered-read and
write-back HBM traffic; the host upcasts the result to fp32.

Window sections are capacity-padded with dummy index 0 so every gather's
descriptor count is static. num_idxs per dma_gather is capped at 1024 by the
ucode's index-read pattern (HW-probed; larger values crash the device), so
each window is gathered in 16 chunks rotated across the 4 SWDGE queues.

Any token overflowing its window's capacity (impossible in practice for
uniform indices: capacity is mean + ~5 sigma) is gathered on the host, so
correctness never depends on the index distribution.
"""
import sys
import numpy as np

sys.path.insert(0, "/opt/trn_rl_repo")

import ml_dtypes

import concourse.bacc as bacc
import concourse.bass as bass
import concourse.mybir as mybir
import concourse.tile as tile
from concourse import bass_utils

N_EMB = 1_000_000
D = 128
N_IDX = 500_000
N_CORES = 8

W_ROWS = 31_250              # rows per window (< 2**15 so int16 indexes work)
WIN_PER_CORE = 4
CORE_ROWS = W_ROWS * WIN_PER_CORE      # 125000 table rows owned per core

# HW-probed limit: dma_gather works at num_idxs=1024 and crashes the device
# at 1152+ (the ucode's index-read pattern tops out at 64 int16 columns per
# partition), so one gather instruction moves at most 1024 rows.
IDX_PER_GATHER = 1024
CHUNKS_PER_WIN = 16
CAP = IDX_PER_GATHER * CHUNKS_PER_WIN  # 16384 token slots per window section
NTILE = WIN_PER_CORE * CHUNKS_PER_WIN  # gather chunks per core
IDX_COLS = IDX_PER_GATHER // 16        # int16 idx columns per partition
GCOLS = -(-IDX_PER_GATHER // 128)      # dst free-dim row groups (cdiv)

DTYPE = mybir.dt.bfloat16
NP_DTYPE = ml_dtypes.bfloat16

_cached = None


def _build():
    global _cached
    if _cached is not None:
        return _cached

    nc = bacc.Bacc(
        "TRN2",
        target_bir_lowering=False,
        debug=False,
        enable_asserts=False,
        num_devices=N_CORES,
        num_swdge_queues=4,
    )
    # int16 window-local indices, 16-wrapped (token i of a chunk at partition
    # i%16, column i//16) and replicated to all 8 gpsimd cores' partitions.
    idx16 = nc.dram_tensor(
        "idx16", [128, NTILE * IDX_COLS], mybir.dt.int16, kind="ExternalInput"
    ).ap()
    wsh = nc.dram_tensor(
        "wsh", [CORE_ROWS, D], DTYPE, kind="ExternalInput"
    ).ap()
    out = nc.dram_tensor(
        "out", [NTILE, 128, GCOLS, D], DTYPE, kind="ExternalOutput"
    ).ap()

    with tile.TileContext(nc) as tc:
        with (
            tc.tile_pool(name="idxp", bufs=1) as idxp,
            tc.tile_pool(name="pool", bufs=3) as pool,
        ):
            idx_all = idxp.tile([128, NTILE * IDX_COLS], mybir.dt.int16)
            nc.sync.dma_start(out=idx_all[:, :], in_=idx16[:, :])
            for k in range(NTILE):
                w = k // CHUNKS_PER_WIN
                g = pool.tile([128, GCOLS, D], DTYPE, tag="g")
                # One instruction gathers this chunk's rows:
                # row i of the chunk -> g[i%128, i//128, :].
                nc.gpsimd.dma_gather(
                    g[:, :, :],
                    wsh[w * W_ROWS:(w + 1) * W_ROWS, :],
                    idx_all[:, k * IDX_COLS:(k + 1) * IDX_COLS],
                    IDX_PER_GATHER,   # num_idxs
                    IDX_PER_GATHER,   # num_idxs_reg: static, all slots valid
                    D,                # elem_size (elements per row)
                    elem_step=D,
                    queue_num=k % 4,
                )
                wb = nc.sync if k % 2 == 0 else nc.scalar
                wb.dma_start(out=out[k], in_=g[:, :, :])

    nc.compile()
    _cached = nc
    return nc


def make_feeds(input, weight):
    """Route tokens to (core, window, slot); build per-core device feeds.

    Returns (in_maps, flat_slot_of_token, host_idx) where flat_slot_of_token
    maps token t to its row in the concatenated device outputs (-1 if the
    token overflowed its window and must be host-gathered from host_idx).
    """
    idx = np.asarray(input).astype(np.int64).ravel()
    assert idx.shape == (N_IDX,)
    w = np.asarray(weight).astype(NP_DTYPE)

    ws = idx // W_ROWS                      # global window id, 0..31
    lo = (idx % W_ROWS).astype(np.int16)    # window-local row

    order = np.argsort(ws, kind="stable")
    ws_sorted = ws[order]
    counts = np.bincount(ws, minlength=N_CORES * WIN_PER_CORE)
    starts = np.zeros(N_CORES * WIN_PER_CORE + 1, dtype=np.int64)
    np.cumsum(counts, out=starts[1:])
    rank = np.arange(N_IDX, dtype=np.int64) - starts[ws_sorted]
    valid = rank < CAP                      # overflow -> host fallback

    slot_global = ws_sorted * CAP + rank    # slot in [32 * CAP) padded space
    lo_slots = np.zeros(N_CORES * WIN_PER_CORE * CAP, dtype=np.int16)
    lo_slots[slot_global[valid]] = lo[order][valid]

    # Wrap-16 + replicate-to-128-partitions feed layout per chunk.
    wrapped = lo_slots.reshape(N_CORES, NTILE, IDX_COLS, 16).transpose(0, 1, 3, 2)
    feed = np.broadcast_to(
        wrapped.reshape(N_CORES, NTILE, 1, 16, IDX_COLS),
        (N_CORES, NTILE, 8, 16, IDX_COLS),
    ).reshape(N_CORES, NTILE, 128, IDX_COLS).transpose(0, 2, 1, 3)
    idx16_feed = np.ascontiguousarray(
        feed.reshape(N_CORES, 128, NTILE * IDX_COLS)
    )

    in_maps = [
        {
            "idx16": idx16_feed[c],
            "wsh": np.ascontiguousarray(w[c * CORE_ROWS:(c + 1) * CORE_ROWS]),
        }
        for c in range(N_CORES)
    ]

    # Device row of slot s (core c, chunk k, i = s % IDX_PER_GATHER):
    # concatenated-out flat row ((c*NTILE + k)*128 + i%128)*GCOLS + i//128
    sg = slot_global
    c_ = sg // (NTILE * IDX_PER_GATHER)
    s_ = sg % (NTILE * IDX_PER_GATHER)
    k_ = s_ // IDX_PER_GATHER
    i_ = s_ % IDX_PER_GATHER
    flat_sorted = ((c_ * NTILE + k_) * 128 + i_ % 128) * GCOLS + i_ // 128
    flat_slot_of_token = np.full(N_IDX, -1, dtype=np.int64)
    flat_slot_of_token[order[valid]] = flat_sorted[valid]
    return in_maps, flat_slot_of_token, idx


def kernel(input, weight, _trace=False, _tmpdir=None):
    nc = _build()
    in_maps, flat_slot, idx = make_feeds(input, weight)

    res = bass_utils.run_bass_kernel_spmd(
        nc,
        in_maps,
        core_ids=list(range(N_CORES)),
        trace=_trace,
        tmpdir=_tmpdir,
    )

    allrows = np.concatenate(
        [
            np.asarray(res.results[c]["out"]).reshape(NTILE * 128 * GCOLS, D)
            for c in range(N_CORES)
        ],
        axis=0,
    )
    missing = flat_slot < 0
    out = allrows[np.where(missing, 0, flat_slot)].astype(np.float32)
    if missing.any():
        wfull = np.asarray(weight, dtype=np.float32)
        out[missing] = wfull[idx[missing]]
    if _trace:
        return out, res
    return out


# revision 10
# speedup vs baseline: 1.1305x; 1.1305x over previous
"""Pair-packed variant: tokens with consecutive table rows share one 512B
descriptor (elem_size=2 rows, overlapping src AP with row stride 1 row),
cutting the Q7-bound per-index work by ~24% on the real input (47.5% of
tokens pair after the in-window sort)."""
import sys
import numpy as np

sys.path.insert(0, "/opt/trn_rl_repo")

import ml_dtypes

import concourse.bacc as bacc
import concourse.bass as bass
import concourse.mybir as mybir
import concourse.tile as tile
from concourse import bass_utils

N_EMB = 1_000_000
D = 128
N_IDX = 500_000
N_CORES = 8

W_ROWS = 31_250
WIN_PER_CORE = 4
CORE_ROWS = W_ROWS * WIN_PER_CORE

CHUNK = 1024                 # idxs per gather (HW cap)
P_CHUNKS = 4                 # pair chunks per window: 4096 pairs = 8192 tokens
S_CHUNKS = 9                 # single chunks per window: 9216 tokens
P_CAP = CHUNK * P_CHUNKS
S_CAP = CHUNK * S_CHUNKS
NP_TILE = WIN_PER_CORE * P_CHUNKS   # 16 pair chunks per core
NS_TILE = WIN_PER_CORE * S_CHUNKS   # 36 single chunks per core
IDX_COLS = CHUNK // 16              # 64
GCOLS = CHUNK // 128                # 8

DTYPE = mybir.dt.bfloat16
NP_DTYPE = ml_dtypes.bfloat16

PAIR_ROWS = N_CORES * NP_TILE * CHUNK * 2   # tokens held in pair region

_cached = None


def _build():
    global _cached
    if _cached is not None:
        return _cached

    nc = bacc.Bacc(
        "TRN2", target_bir_lowering=False, debug=False, enable_asserts=False,
        num_devices=N_CORES, num_swdge_queues=4,
    )
    tot_cols = (NP_TILE + NS_TILE) * IDX_COLS
    idx16 = nc.dram_tensor(
        "idx16", [128, tot_cols], mybir.dt.int16, kind="ExternalInput"
    ).ap()
    wsh = nc.dram_tensor("wsh", [CORE_ROWS, D], DTYPE, kind="ExternalInput").ap()
    outp = nc.dram_tensor(
        "outp", [NP_TILE, 128, GCOLS, 2 * D], DTYPE, kind="ExternalOutput"
    ).ap()
    outs = nc.dram_tensor(
        "outs", [NS_TILE, 128, GCOLS, D], DTYPE, kind="ExternalOutput"
    ).ap()

    with tile.TileContext(nc) as tc:
        with (
            tc.tile_pool(name="idxp", bufs=1) as idxp,
            tc.tile_pool(name="poolp", bufs=3) as poolp,
            tc.tile_pool(name="pools", bufs=3) as pools,
        ):
            idx_all = idxp.tile([128, tot_cols], mybir.dt.int16)
            nc.sync.dma_start(out=idx_all[:, :], in_=idx16[:, :])
            q = 0
            for w in range(WIN_PER_CORE):
                base = w * (P_CHUNKS + S_CHUNKS) * IDX_COLS
                # Overlapping src AP: element row stride D, width 2*D ->
                # index r reads table rows r and r+1 in one descriptor.
                pair_src = bass.AP(
                    wsh.tensor, w * W_ROWS * D,
                    [[D, W_ROWS - 1], [1, 2 * D]],
                )
                for pc in range(P_CHUNKS):
                    g = poolp.tile([128, GCOLS, 2 * D], DTYPE, tag="gp")
                    nc.gpsimd.dma_gather(
                        g[:, :, :], pair_src,
                        idx_all[:, base + pc * IDX_COLS:base + (pc + 1) * IDX_COLS],
                        CHUNK, CHUNK, 2 * D, elem_step=D, queue_num=q % 4,
                    )
                    wb = nc.sync if q % 2 == 0 else nc.scalar
                    wb.dma_start(out=outp[w * P_CHUNKS + pc], in_=g[:, :, :])
                    q += 1
                sbase = base + P_CHUNKS * IDX_COLS
                for sc in range(S_CHUNKS):
                    g = pools.tile([128, GCOLS, D], DTYPE, tag="gs")
                    nc.gpsimd.dma_gather(
                        g[:, :, :],
                        wsh[w * W_ROWS:(w + 1) * W_ROWS, :],
                        idx_all[:, sbase + sc * IDX_COLS:sbase + (sc + 1) * IDX_COLS],
                        CHUNK, CHUNK, D, elem_step=D, queue_num=q % 4,
                    )
                    wb = nc.sync if q % 2 == 0 else nc.scalar
                    wb.dma_start(out=outs[w * S_CHUNKS + sc], in_=g[:, :, :])
                    q += 1

    nc.compile()
    _cached = nc
    return nc


def _wrap16(arr):
    """[n_chunks, CHUNK] int16 -> [128, n_chunks*IDX_COLS] wrap-16 feed."""
    n = arr.shape[0]
    w = arr.reshape(n, IDX_COLS, 16).transpose(0, 2, 1)  # [n,16,cols]
    f = np.broadcast_to(w.reshape(n, 1, 16, IDX_COLS), (n, 8, 16, IDX_COLS))
    return np.ascontiguousarray(
        f.reshape(n, 128, IDX_COLS).transpose(1, 0, 2).reshape(128, n * IDX_COLS)
    )


def make_feeds(input, weight):
    idx = np.asarray(input).astype(np.int64).ravel()
    assert idx.shape == (N_IDX,)
    w = np.asarray(weight).astype(NP_DTYPE)

    ws = idx // W_ROWS
    lo = (idx % W_ROWS).astype(np.int64)
    order = np.argsort(ws * W_ROWS + lo, kind="stable")  # sort by (window, lo)
    ws_s, lo_s = ws[order], lo[order]

    n = N_IDX
    same_win_next = np.zeros(n, dtype=bool)
    same_win_next[:-1] = (ws_s[1:] == ws_s[:-1]) & (lo_s[1:] == lo_s[:-1] + 1)
    # greedy chain pairing: chain starts where prev link is absent
    link_prev = np.zeros(n, dtype=bool)
    link_prev[1:] = same_win_next[:-1]
    start = ~link_prev
    chain_first = np.maximum.accumulate(np.where(start, np.arange(n), -1))
    pos = np.arange(n) - chain_first
    head = (pos % 2 == 0) & same_win_next
    tail = np.zeros(n, dtype=bool)
    tail[1:] = head[:-1]
    single = ~head & ~tail

    # per-window ranks
    win_of = ws_s
    pair_rank = np.cumsum(head) - 1
    pair_start = np.zeros(33, dtype=np.int64)
    pair_cnt = np.bincount(win_of[head], minlength=32)
    np.cumsum(pair_cnt, out=pair_start[1:])
    pr = pair_rank - pair_start[win_of]          # pair rank within window

    # demote overflowing pairs to singles
    over = head & (pr >= P_CAP)
    over_tail = np.zeros(n, dtype=bool)
    over_tail[1:] = over[:-1]
    head &= ~over
    tail &= ~over_tail
    single |= over | over_tail

    sr_all = np.cumsum(single) - 1
    s_start = np.zeros(33, dtype=np.int64)
    s_cnt = np.bincount(win_of[single], minlength=32)
    np.cumsum(s_cnt, out=s_start[1:])
    sr = sr_all - s_start[win_of]                # single rank within window
    valid_s = single & (sr < S_CAP)
    fallback = single & ~valid_s

    # idx feeds
    pair_slots = np.zeros(32 * P_CAP, dtype=np.int16)
    pair_slots[win_of[head] * P_CAP + pr[head]] = lo_s[head].astype(np.int16)
    single_slots = np.zeros(32 * S_CAP, dtype=np.int16)
    single_slots[win_of[valid_s] * S_CAP + sr[valid_s]] = lo_s[valid_s].astype(np.int16)

    in_maps = []
    for c in range(N_CORES):
        pw = pair_slots[c * 4 * P_CAP:(c + 1) * 4 * P_CAP].reshape(4, P_CHUNKS, CHUNK)
        sw = single_slots[c * 4 * S_CAP:(c + 1) * 4 * S_CAP].reshape(4, S_CHUNKS, CHUNK)
        # interleave per window: [w0 pairs, w0 singles, w1 pairs, ...]
        chunks = []
        for wi in range(4):
            chunks.append(pw[wi])
            chunks.append(sw[wi])
        feed = _wrap16(np.concatenate(chunks, axis=0))
        in_maps.append({
            "idx16": feed,
            "wsh": np.ascontiguousarray(w[c * CORE_ROWS:(c + 1) * CORE_ROWS]),
        })

    # flat row mapping (allrows = [all cores outp rows][all cores outs rows])
    flat = np.full(n, -1, dtype=np.int64)
    c_ = win_of // 4
    wl = win_of % 4
    # pair head at sorted index j, tail at j+1; both share the head's slot
    hidx = np.where(head)[0]
    qq = pr[hidx]
    k = wl[hidx] * P_CHUNKS + qq // CHUNK
    i = qq % CHUNK
    prow = ((c_[hidx] * NP_TILE + k) * 128 + i % 128) * GCOLS + i // 128
    flat[hidx] = prow * 2
    flat[hidx + 1] = prow * 2 + 1
    base_s = N_CORES * NP_TILE * CHUNK * 2
    k = wl[valid_s] * S_CHUNKS + sr[valid_s] // CHUNK
    i = sr[valid_s] % CHUNK
    flat[valid_s] = base_s + ((c_[valid_s] * NS_TILE + k) * 128 + i % 128) * GCOLS + i // 128

    flat_tok = np.full(n, -1, dtype=np.int64)
    flat_tok[order] = flat
    return in_maps, flat_tok, idx


def kernel(input, weight, _trace=False, _tmpdir=None):
    nc = _build()
    in_maps, flat_slot, idx = make_feeds(input, weight)
    res = bass_utils.run_bass_kernel_spmd(
        nc, in_maps, core_ids=list(range(N_CORES)), trace=_trace, tmpdir=_tmpdir,
    )
    prows = np.concatenate(
        [np.asarray(res.results[c]["outp"]).reshape(-1, D) for c in range(N_CORES)], axis=0)
    srows = np.concatenate(
        [np.asarray(res.results[c]["outs"]).reshape(-1, D) for c in range(N_CORES)], axis=0)
    allrows = np.concatenate([prows, srows], axis=0)
    missing = flat_slot < 0
    out = allrows[np.where(missing, 0, flat_slot)].astype(np.float32)
    if missing.any():
        wfull = np.asarray(weight, dtype=np.float32)
        out[missing] = wfull[idx[missing]]
    if _trace:
        return out, res
    return out
